# revision 6
# baseline (speedup 1.0000x reference)
"""Trainium2 Bass kernel for BalancedSkipGramModel scoring (embedding_lookup).

Computation (per element e = (b,l,s) with s indexing the 5 pos + 25 neg
context slots of walk position (b,l)):

    score[e]  = sum_d node[ctx_row[e], d] * node[walk_row[b,l], d]
                      * sigmoid(rel[4*wt[b,l] + ct[e], d])
    ptype[e]  = 16*k(s) + 4*wt[b,l] + ct[e]

Strategy (8 NeuronCores, batch-parallel, 32 batches each):
  * walk embeddings (2400 rows/core) gathered block-aligned via
    indirect_dma_start; sigmoid(rel) computed on device; a combined
    wg table wg[(r,pt)] = walk_e[r] * sig[4*wt(r)+pt] (9728 rows) is built
    on device and written to DRAM scratch.
  * the 72000 ctx elements are sorted (host-side index prep) by 32K-row
    windows of the node table so the big node gather runs as 31
    dma_gather ops with int16 window-local indices (~2.7k rows each, one
    Q7 instruction per window instead of one per 128 rows).
  * per element, the matching wg row is gathered from the DRAM scratch
    with the same list order, so a single f32 multiply + free-dim reduce
    on DVE produces the scores; the host undoes the sort permutation.

The module compiles the (input-independent) program once and caches it.
"""

import numpy as np
from contextlib import ExitStack

import concourse.bass as bass
import concourse.tile as tile
from concourse import bacc, mybir
from concourse.bass import IndirectOffsetOnAxis
from concourse.bass_utils import run_bass_kernel_spmd
from concourse.library_config import mlp as _mlp_lib

# ---- problem constants (hardcoded per contract) ----
NODE_NUM = 1_000_000
T = 4
DIM = 128
K = 5
M = 5
B = 256
LK = 75
NCORES = 8
BPC = B // NCORES            # 32 batches per core
RROWS = BPC * LK             # 2400 (b,l) rows per core
NBLK = (RROWS + 127) // 128  # 19 row blocks
RPAD = NBLK * 128            # 2432
S = K + K * M                # 30 ctx elements per row
NELEM = RROWS * S            # 72000 elements per core
WIN = 32768                  # int16-addressable window of the node table
NWIN = (NODE_NUM + WIN - 1) // WIN   # 31
CAP = 2688                   # padded element capacity per window (mult of 128)
SLOTW = CAP // 128           # 21 sbuf slots per window
NSLOT = NWIN * SLOTW         # 651 score slots
WPC = 2                      # windows per processing chunk
NCHUNK = (NWIN + WPC - 1) // WPC
NWG = RPAD * T               # 9728 wg rows
GGRP = 4                     # walk blocks per wg-build group

f32 = mybir.dt.float32
i32 = mybir.dt.int32
i16 = mybir.dt.int16

_CACHED = {}


def _build_nc(cap):
    slotw = cap // 128
    nslot = NWIN * slotw
    icols = cap // 16  # idx cols per window in the wrapped int16 layout

    nc = bacc.Bacc("TRN2", target_bir_lowering=False, debug=False,
                   num_devices=NCORES)
    node_t = nc.dram_tensor("node_emb", [NODE_NUM, DIM], f32,
                            kind="ExternalInput")
    rel_t = nc.dram_tensor("rel_emb", [T * T, DIM], f32, kind="ExternalInput")
    widx_t = nc.dram_tensor("walk_idx", [128, NBLK], i32, kind="ExternalInput")
    wt_t = nc.dram_tensor("walk_tp", [128, NBLK], i32, kind="ExternalInput")
    ptnt_t = nc.dram_tensor("ptnt", [128, NBLK * S], i32, kind="ExternalInput")
    wrep_t = nc.dram_tensor("wrep", [128, NBLK * S], i32, kind="ExternalInput")
    k16_t = nc.dram_tensor("k16", [128, NBLK * S], i32, kind="ExternalInput")
    nidx_t = nc.dram_tensor("node_lists", [128, NWIN * icols], i16,
                            kind="ExternalInput")
    gidx_t = nc.dram_tensor("wg_lists", [128, NWIN * icols], i16,
                            kind="ExternalInput")
    scores_t = nc.dram_tensor("scores", [128, nslot], f32,
                              kind="ExternalOutput")
    tout_t = nc.dram_tensor("tout", [128, NBLK * S], i32,
                            kind="ExternalOutput")

    AL = mybir.AluOpType

    with tile.TileContext(nc) as tc, ExitStack() as ctx:
        nc.gpsimd.load_library(_mlp_lib)

        dpool = ctx.enter_context(
            tc.tile_pool(name="dram", bufs=1, space="DRAM"))
        wg_dram = dpool.tile([NWG, DIM], f32)
        sig_dram = dpool.tile([T, T * DIM], f32)  # [4,512]: 4 sigmoid rows/entry

        cpool = ctx.enter_context(tc.tile_pool(name="const", bufs=1))

        nidx_sb = cpool.tile([128, NWIN * icols], i16)
        nc.sync.dma_start(nidx_sb[:], nidx_t.ap())
        gidx_sb = cpool.tile([128, NWIN * icols], i16)
        nc.sync.dma_start(gidx_sb[:], gidx_t.ap())
        widx_sb = cpool.tile([128, NBLK], i32)
        nc.sync.dma_start(widx_sb[:], widx_t.ap())
        wt_sb = cpool.tile([128, NBLK], i32)
        nc.sync.dma_start(wt_sb[:], wt_t.ap())
        scores_sb = cpool.tile([128, nslot], f32)

        # pair-type outputs: tout = 16*k + 4*wt + ct
        ptnt_sb = cpool.tile([128, NBLK * S], i32)
        nc.sync.dma_start(ptnt_sb[:], ptnt_t.ap())
        wrep_sb = cpool.tile([128, NBLK * S], i32)
        nc.sync.dma_start(wrep_sb[:], wrep_t.ap())
        k16_sb = cpool.tile([128, NBLK * S], i32)
        nc.sync.dma_start(k16_sb[:], k16_t.ap())
        tp_sb = cpool.tile([128, NBLK * S], i32)
        nc.vector.tensor_scalar(tp_sb[:], wrep_sb[:], 4, None, AL.mult)
        nc.vector.tensor_tensor(tp_sb[:], tp_sb[:], ptnt_sb[:], AL.add)
        tout_sb = cpool.tile([128, NBLK * S], i32)
        nc.vector.tensor_tensor(tout_sb[:], tp_sb[:], k16_sb[:], AL.add)
        nc.sync.dma_start(tout_t.ap(), tout_sb[:])

        # sigmoid(rel) -> DRAM scratch
        rel_sb = cpool.tile([T * T, DIM], f32)
        nc.sync.dma_start(rel_sb[:], rel_t.ap())
        sig_sb = cpool.tile([T * T, DIM], f32)
        nc.scalar.activation(sig_sb[:], rel_sb[:],
                             mybir.ActivationFunctionType.Sigmoid)
        nc.sync.dma_start(sig_dram[:], sig_sb[:])

        # ---- stage 1: build wg table (walk_e * sig4) in groups ----
        ngrp = (NBLK + GGRP - 1) // GGRP
        wgv = wg_dram[:].rearrange("(p r) d -> p r d", p=128)
        with ExitStack() as s1:
            wkp = s1.enter_context(tc.tile_pool(name="wk", bufs=2))
            sgp = s1.enter_context(tc.tile_pool(name="sg4", bufs=2))
            wgp = s1.enter_context(tc.tile_pool(name="wg", bufs=2))
            for g in range(ngrp):
                b0 = g * GGRP
                gsz = min(GGRP, NBLK - b0)
                # HW indirect gather needs flat 2D dest/src APs
                wk_g = wkp.tile([128, gsz * DIM], f32)
                sg_g = sgp.tile([128, gsz * T * DIM], f32)
                for i in range(gsz):
                    b = b0 + i
                    nc.gpsimd.indirect_dma_start(
                        out=wk_g[:, i * DIM:(i + 1) * DIM], out_offset=None,
                        in_=node_t.ap(),
                        in_offset=IndirectOffsetOnAxis(
                            ap=widx_sb[:, b:b + 1], axis=0),
                    )
                    nc.gpsimd.indirect_dma_start(
                        out=sg_g[:, i * T * DIM:(i + 1) * T * DIM],
                        out_offset=None,
                        in_=sig_dram[:],
                        in_offset=IndirectOffsetOnAxis(
                            ap=wt_sb[:, b:b + 1], axis=0),
                    )
                wg_g = wgp.tile([128, gsz, T, DIM], f32)
                nc.vector.tensor_tensor(
                    wg_g[:], sg_g[:],
                    wk_g[:].rearrange("p (g d) -> p g d", g=gsz)
                    .unsqueeze(2).to_broadcast([128, gsz, T, DIM]),
                    AL.mult)
                nc.sync.dma_start(
                    wgv[:, b0 * T:(b0 + gsz) * T, :], wg_g[:])

        # ---- stage 2: windowed node gather + wg gather + mul-reduce ----
        epool = ctx.enter_context(tc.tile_pool(name="e", bufs=2))
        wpool = ctx.enter_context(tc.tile_pool(name="w", bufs=2))
        upool = ctx.enter_context(tc.tile_pool(name="u", bufs=2))
        for c in range(NCHUNK):
            w0 = c * WPC
            wc = min(WPC, NWIN - w0)
            e_c = epool.tile([128, wc * slotw, DIM], f32)
            for j in range(wc):
                w = w0 + j
                hi = min((w + 1) * WIN, NODE_NUM)
                nc.gpsimd.dma_gather(
                    e_c[:, j * slotw:(j + 1) * slotw, :],
                    node_t.ap()[w * WIN:hi, :],
                    nidx_sb[:, w * icols:(w + 1) * icols],
                    cap, cap, DIM, single_packet=False,
                )
            wg_c = wpool.tile([128, wc * slotw, DIM], f32)
            nc.gpsimd.dma_gather(
                wg_c[:], wg_dram[:],
                gidx_sb[:, w0 * icols:(w0 + wc) * icols],
                wc * cap, wc * cap, DIM, single_packet=False,
            )
            u_c = upool.tile([128, wc * slotw, DIM], f32)
            nc.vector.tensor_tensor(u_c[:], e_c[:], wg_c[:], AL.mult)
            nc.vector.tensor_reduce(
                scores_sb[:, w0 * slotw:(w0 + wc) * slotw],
                u_c[:], mybir.AxisListType.X, AL.add)
        nc.sync.dma_start(scores_t.ap(), scores_sb[:])

    nc.compile()
    return nc


def _get_nc(cap):
    if cap not in _CACHED:
        _CACHED[cap] = _build_nc(cap)
    return _CACHED[cap]


def _wrap_idx(lst):
    """int16 list -> [128, len/16] wrapped (j -> [j%16, j//16]), replicated
    into all 8 groups of 16 partitions."""
    a = np.asarray(lst, np.int16).reshape(-1, 16).T  # [16, cols]
    return np.tile(a, (8, 1))


def _pack_rows(a):
    """[RPAD, C] row-major -> [128, NBLK*C] block layout
    (out[p, b*C + c] = a[b*128 + p, c])."""
    c = a.shape[1]
    return (a.reshape(NBLK, 128, c).transpose(1, 0, 2)
            .reshape(128, NBLK * c))


def _prep_core(walk, pos, neg, walk_type, pos_type, neg_type, cap):
    """Host-side index prep for one core. Returns (in_map, meta)."""
    wk = walk.reshape(-1).astype(np.int64)
    ps = pos.reshape(RROWS, K).astype(np.int64)
    ng = neg.reshape(RROWS, K * M).astype(np.int64)
    wt = walk_type.reshape(-1).astype(np.int64)
    pt = pos_type.reshape(RROWS, K).astype(np.int64)
    nt = neg_type.reshape(RROWS, K * M).astype(np.int64)

    pad = RPAD - RROWS
    wk_p = np.concatenate([wk, np.zeros(pad, np.int64)])
    wt_p = np.concatenate([wt, np.zeros(pad, np.int64)])

    elem_node = np.concatenate([ps, ng], 1)        # [RROWS, 30]
    elem_ct = np.concatenate([pt, nt], 1)          # [RROWS, 30]

    flat_node = elem_node.reshape(-1)              # [72000]
    win = (flat_node >> 15).astype(np.int64)
    order = np.argsort(win, kind="stable")         # element ids, window-sorted
    counts = np.bincount(win, minlength=NWIN)
    if counts.max() > cap:
        return None, None

    local = (flat_node & (WIN - 1)).astype(np.int16)
    # wg row id, partition-major: r = b*128+p  ->  p*(NBLK*T) + b*T + pt
    rr = np.repeat(np.arange(RROWS), S)
    p_of = rr % 128
    b_of = rr // 128
    wg_local = (p_of * (NBLK * T) + b_of * T
                + elem_ct.reshape(-1)).astype(np.int16)

    starts = np.zeros(NWIN, np.int64)
    starts[1:] = np.cumsum(counts)[:-1]
    nlist = np.zeros(NWIN * cap, np.int16)
    glist = np.zeros(NWIN * cap, np.int16)
    sw = win[order]
    ranks = np.arange(NELEM) - starts[sw]
    lpos = sw * cap + ranks                        # list position per sorted elt
    nlist[lpos] = local[order]
    glist[lpos] = wg_local[order]

    # pair-type packing for the block-layout t_out path
    ct_p = np.concatenate([elem_ct, np.zeros((pad, S), np.int64)])
    kpat = np.array([16 * k for k in range(K)]
                    + [16 * (s // M) for s in range(K * M)], np.int64)

    in_map = {
        "walk_idx": wk_p.reshape(NBLK, 128).T.astype(np.int32).copy(),
        "walk_tp": wt_p.reshape(NBLK, 128).T.astype(np.int32).copy(),
        "ptnt": _pack_rows(ct_p).astype(np.int32).copy(),
        "wrep": _pack_rows(np.repeat(wt_p[:, None], S, 1)).astype(np.int32).copy(),
        "k16": _pack_rows(np.tile(kpat, (RPAD, 1))).astype(np.int32).copy(),
        "node_lists": _wrap_idx(nlist).copy(),
        "wg_lists": _wrap_idx(glist).copy(),
    }
    meta = {"order": order, "lpos": lpos}
    return in_map, meta


def kernel(**inputs):
    walk = np.asarray(inputs["walk"])
    pos = np.asarray(inputs["pos"])
    neg = np.asarray(inputs["neg"])
    walk_type = np.asarray(inputs["walk_type"])
    pos_type = np.asarray(inputs["pos_type"])
    neg_type = np.asarray(inputs["neg_type"])
    node = np.ascontiguousarray(
        np.asarray(inputs["node_embedding"], dtype=np.float32))
    rel = np.ascontiguousarray(
        np.asarray(inputs["relationship_embedding"], dtype=np.float32))
    idt = walk.dtype

    cap = CAP
    while True:
        in_maps, metas = [], []
        ok = True
        for c in range(NCORES):
            sl = slice(c * BPC, (c + 1) * BPC)
            im, meta = _prep_core(walk[sl], pos[sl], neg[sl], walk_type[sl],
                                  pos_type[sl], neg_type[sl], cap)
            if im is None:
                ok = False
                break
            im["node_emb"] = node
            im["rel_emb"] = rel
            in_maps.append(im)
            metas.append(meta)
        if ok:
            break
        cap += 256  # rare: a window overflowed the padded capacity

    nc = _get_nc(cap)
    res = run_bass_kernel_spmd(nc, in_maps, list(range(NCORES)))

    slotw = cap // 128
    pos_scores, neg_scores, pos_tp, neg_tp = [], [], [], []
    for c in range(NCORES):
        out = res.results[c]
        dev_scores = out["scores"]                 # [128, NWIN*slotw]
        meta = metas[c]
        lpos = meta["lpos"]
        part = lpos % 128
        # within-window rank -> slot: slot = w*slotw + rank//128
        w = lpos // cap
        rank = lpos - w * cap
        slot = w * slotw + rank // 128
        s_sorted = dev_scores[part, slot]
        s_flat = np.empty(NELEM, np.float32)
        s_flat[meta["order"]] = s_sorted
        s2 = s_flat.reshape(RROWS, S)
        pos_scores.append(s2[:, :K])
        neg_scores.append(s2[:, K:])

        tout = out["tout"].reshape(128, NBLK, S).transpose(1, 0, 2)
        tout = tout.reshape(RPAD, S)[:RROWS]
        pos_tp.append(tout[:, :K])
        neg_tp.append(tout[:, K:])

    pos_score = np.concatenate(pos_scores).reshape(-1).astype(np.float32)
    neg_score = np.concatenate(neg_scores).reshape(-1).astype(np.float32)
    pos_pair = np.concatenate(pos_tp).reshape(-1).astype(idt)
    neg_pair = np.concatenate(neg_tp).reshape(-1).astype(idt)
    return (pos_score, neg_score, pos_pair, neg_pair)
